# revision 19
# baseline (speedup 1.0000x reference)
"""Trainium2 Bass kernel for nn_Cross_modal_ContrastiveLoss6.

Math: the reference loss only depends on per-class means of the two
modalities (every entry of the N x N distance matrix is determined by the
class pair), so the whole computation reduces to:

  1. raw per-class segment sums R[c,d], T[c,d]  (memory-bound)
  2. the three 128x128 class Gram matrices P1 = R R^T, P2 = T T^T, P3 = R T^T
  3. tiny 128x128 class-pair loss math with the class counts

Device strategy (8 cores, feature/d-sharded so no cross-core collective is
needed): core k takes columns [256k, 256k+256) of both modal tensors and
computes the full-N segment sums for its d-chunk with one-hot matmuls on
the PE.  The data ships as fp8 e4m3 (1 byte/elem): the loss averages the
quantization noise over ~32 samples/class x 2048 dims, so the end-to-end
error stays ~6e-4 -- far inside the 2e-2 gate -- while HBM traffic drops
4x vs fp32 (2 MiB/core).  Both modals are interleaved per 128-sample block
([x1_b | x2_b] = 512 fp8 cols) and two sample blocks are contracted per
matmul with DoubleRow fp8 (2 one-hot weights/PE cell), halving the weight
loads and instruction count vs single-pump.  Even pairs accumulate in the
first PSUM bank, odd pairs in the second, so consecutive matmuls pipeline;
the banks ship separately in fp16 and the host adds them.  The one-hot for
the first three block pairs rides at the head of the first x transfer so
the PE starts without waiting for the targets round-trip; the rest is
generated on-device (gpsimd iota + one DVE is_equal per chunk).  The host
forms the three Grams and does the count scaling + sqrt/relu/weighted mean
(<0.1% of the FLOPs) in float64.
"""

import numpy as np
import ml_dtypes

import concourse.bacc as bacc
import concourse.bass as bass
import concourse.mybir as mybir
from concourse.bass_utils import run_bass_kernel_spmd

N = 4096
D = 2048
C = 128
MARGIN = 0.5
NCORES = 8
DCHUNK = D // NCORES          # 256 feature columns per core
P = 128                       # partitions / sample-block size
NB = N // P                   # 32 sample blocks
W = 2 * DCHUNK                # 512 interleaved fp8 cols per block (x1|x2)
NPAIR = NB // 2               # 16 DoubleRow block pairs
PW = 2 * W                    # 1024 fp8 cols per pair
OH0_PAIRS = 3                 # pairs with host-precomputed one-hot
HEADC = OH0_PAIRS * 2 * C     # 768 head cols: one-hot for pairs 0..2
XCOLS = HEADC + NPAIR * PW    # total x columns per partition (17152 B)
# x-DMA chunking in block *pairs*: chunk c covers pairs [off[c], off[c+1]).
# Chunk 0 additionally carries the 768-col one-hot head.  Each dma_start
# costs ~0.7us of HWDGE issue (128 descriptors) and bigger chunks move
# bigger descriptors, so the middle chunks run near the 358 GB/s HBM limit
# while the small head/tail chunks shorten the first-data and last-chunk
# completion latencies.
PAIR_OFF = [0, 1, 3, 7, 11, 15, 16]
NCHUNK = len(PAIR_OFF) - 1
# Queue split: a chunk's completion semaphore only fires once the SLOWEST
# of the 16 SDMA engines finishes it, and with a deep ring backlog the
# engine spread grows to ~3us.  The chunks that gate the PE's start
# (0, 1) and finish (5) go on the sync ring, which stays shallow; the
# mid-stream bulk rides the scalar ring where completion spread is hidden
# behind the PE's own pace.
SYNC_CHUNKS = (0, 1, 3, 5)    # chunk c on sync ring; others on scalar ring

F32 = mybir.dt.float32
F16 = mybir.dt.float16
I32 = mybir.dt.int32
BF16 = mybir.dt.bfloat16
F8 = mybir.dt.float8e4
NPF8 = ml_dtypes.float8_e4m3

_PROGRAM = None


def _chunk_cols(c):
    """x-tensor column range covered by chunk c (chunk 0 includes the head)."""
    lo = 0 if c == 0 else HEADC + PAIR_OFF[c] * PW
    hi = HEADC + PAIR_OFF[c + 1] * PW
    return lo, hi


def _build_program() -> bass.Bass:
    """Raw-bass program (no TileContext): 5 engine streams.

    sync ring:   [oh-head|pair0] + chunks 2,4 -> bank-a output DMA
    scalar ring: targets + chunks 1,3,5 -> bank-b output DMA
    gpsimd:      iota row for the one-hot compare
    tensor:      HAM warmup, then one DoubleRow matmul per block pair
    vector:      per-chunk one-hot is_equal, two PSUM->fp16 casts
    """
    nc = bass.Bass()

    # tgt[p, b] = targets[b*128 + p]; x cols [0:768] = one-hot for pairs
    # 0..2 ([pr, i, c] layout), cols [768:] = pair pr at 768+pr*1024 with
    # [i, n] layout, n in [x1_b | x2_b] -- packed host-side.
    tgt_in = nc.declare_dram_parameter("tgt", [P, NB], F32, isOutput=False)
    x_in = nc.declare_dram_parameter("x", [P, XCOLS], F8, isOutput=False)
    # sums[:, 0:512] = bank a (even pairs, [R|T]), [:, 512:1024] = bank b
    sums_out = nc.declare_dram_parameter("sums", [P, 1024], F16, isOutput=True)

    import contextlib

    with contextlib.ExitStack() as stack:
        oh_t = stack.enter_context(nc.sbuf_tensor([P, NPAIR * 2 * C], F8))
        tgt_t = stack.enter_context(nc.sbuf_tensor([P, NB], F32))
        iota_t = stack.enter_context(nc.sbuf_tensor([P, C], I32))
        x_t = stack.enter_context(nc.sbuf_tensor([P, XCOLS], F8))
        warm_t = stack.enter_context(nc.sbuf_tensor([P, 136], BF16))
        out_t = stack.enter_context(nc.sbuf_tensor([P, 1024], F16))
        # One PSUM tensor spanning two banks: even pairs accumulate in
        # [:, 0:512] (bank a), odd pairs in [:, 512:1024] (bank b), so
        # consecutive matmuls hit different banks and pipeline.
        psum = stack.enter_context(nc.psum_tensor([P, 1024], F32))
        psum_warm = stack.enter_context(nc.psum_tensor([P, 8], F32))

        def sem(name):
            return stack.enter_context(nc.semaphore(name))

        clr_sem = sem("clr_done")
        tgt_sem = sem("tgt_dma")
        iota_sem = sem("iota_gen")
        oh_gen = sem("oh_gen")
        x_sems = [sem(f"x_dma_{j}") for j in range(NCHUNK)]
        pe_done = sem("pe_done")
        vec_a = sem("vec_a")
        vec_b = sem("vec_b")
        dma_out = sem("dma_out")

        # Raw-bass semaphores are NOT cleared by the framework preamble;
        # stale values from whatever ran on the core before would satisfy
        # our waits early.  Clear them, then fence with the NRT pseudo
        # barrier (safe while bass sems are still being cleared).
        all_sems = (
            [clr_sem, tgt_sem, iota_sem, oh_gen]
            + x_sems
            + [pe_done, vec_a, vec_b, dma_out]
        )
        nums = sorted(h.num for h in all_sems)
        assert nums == list(range(nums[0], nums[0] + len(nums))), nums
        sem_range = range(nums[0], nums[-1] + 1)
        nc.gpsimd.dma_reset(sem_range)
        nc.gpsimd.sem_clear(sem_range).then_inc(clr_sem, 1)
        # init the PE-warmup scratch (hidden under the gpsimd clears; also
        # keeps the simulator's uninitialized-read check happy)
        nc.vector.memset(warm_t[:], 0)
        # Prefetch the transfers that gate the PE start (one-hot head +
        # first three block pairs, and the targets) BEFORE the pseudo
        # barrier: their ~2us HBM completion round-trips then overlap the
        # barrier, block entry, and PE warmup instead of stalling the PE.
        # The clr_sem wait orders the issues after the semaphore clears.
        for c in (0, 1):
            lo, hi = _chunk_cols(c)
            nc.sync.wait_ge(clr_sem, 1)
            nc.sync.dma_start(out=x_t[:, lo:hi], in_=x_in[:, lo:hi]).then_inc(
                x_sems[c], 16
            )
        nc.scalar.wait_ge(clr_sem, 1)
        nc.scalar.dma_start(out=tgt_t[:], in_=tgt_in[:]).then_inc(tgt_sem, 16)
        nc._nrt_pseudo_barrier()

        def lhsT_ap(pr):
            # one-hot pair pr: [K=128, Ko=2 (step 128), C=128] -- from the
            # x head for precomputed pairs, else from the DVE-written tile
            if pr < OH0_PAIRS:
                return bass.AP(x_t, pr * 2 * C, [[XCOLS, P], [C, 2], [1, C]])
            return bass.AP(oh_t, pr * 2 * C, [[NPAIR * 2 * C, P], [C, 2], [1, C]])

        def rhs_ap(pr):
            # x pair pr: [K=128, Ko=2 (step 512), n=512]
            return bass.AP(x_t, HEADC + pr * PW, [[XCOLS, P], [W, 2], [1, W]])

        # no_gpsimd_drain: skip the ~5us GpSimd DGE drain at block exit; the
        # block-exit engine drains + barrier fence everything that remains.
        with nc.Block(no_gpsimd_drain=True) as block:

            @block.gpsimd
            def _(gpsimd: bass.BassEngine):
                gpsimd.iota(
                    iota_t[:], [[1, C]], channel_multiplier=0
                ).then_inc(iota_sem, 1)

            @block.sync
            def _(sync: bass.BassEngine):
                for c in SYNC_CHUNKS:
                    if c < 2:
                        continue  # prefetched in the preamble
                    lo, hi = _chunk_cols(c)
                    sync.dma_start(
                        out=x_t[:, lo:hi], in_=x_in[:, lo:hi]
                    ).then_inc(x_sems[c], 16)
                sync.wait_ge(vec_a, 1)
                # no wait on dma_out: the NEFF exit sequence (engine drains +
                # DGE queue drain) runs concurrently with the output write's
                # HBM receipt, hiding ~2us of completion latency.
                sync.dma_start(
                    out=sums_out[:, 0:512], in_=out_t[:, 0:512]
                ).then_inc(dma_out, 16)

            @block.scalar
            def _(scalar: bass.BassEngine):
                for c in range(NCHUNK):
                    if c in SYNC_CHUNKS:
                        continue
                    lo, hi = _chunk_cols(c)
                    scalar.dma_start(
                        out=x_t[:, lo:hi], in_=x_in[:, lo:hi]
                    ).then_inc(x_sems[c], 16)
                scalar.wait_ge(vec_b, 1)
                scalar.dma_start(
                    out=sums_out[:, 512:1024], in_=out_t[:, 512:1024]
                ).then_inc(dma_out, 16)

            @block.tensor
            def _(tensor: bass.BassEngine):
                # Keep the PE HAM activity window busy (junk matmuls on
                # zeroed scratch) while the first DMA chunks land, so the
                # ~3.4us warmup clock runs from block entry instead of from
                # first data.
                for _ in range(8):
                    nc.tensor.matmul(
                        psum_warm[:],
                        warm_t[:, 0:128],
                        warm_t[:, 128:136],
                        start=True,
                        stop=True,
                    )
                for c in range(NCHUNK):
                    if c >= 2:
                        tensor.wait_ge(oh_gen, c - 1)
                    tensor.wait_ge(x_sems[c], 16)
                    for pr in range(PAIR_OFF[c], PAIR_OFF[c + 1]):
                        half = slice(0, 512) if pr % 2 == 0 else slice(512, 1024)
                        nc.tensor.matmul(
                            psum[:, half],
                            lhsT_ap(pr),
                            rhs_ap(pr),
                            start=(pr < 2),
                            stop=(pr >= NPAIR - 2),
                            perf_mode=mybir.MatmulPerfMode.DoubleRow,
                        )
                # drain makes sure the last matmul's PSUM writes have landed
                # before the DVE reads them.
                tensor.drain().then_inc(pe_done, 1)

            @block.vector
            def _(vector: bass.BassEngine):
                # One is_equal per x-chunk (from chunk 2 on; pairs 0..2 ship
                # precomputed) builds that chunk's one-hot pairs in [pr,i,c]
                # layout: oh[p, pr, i, c] = (targets[(2pr+i)*128+p] == c),
                # via broadcast APs (iota repeated per (pr, i), target column
                # repeated per c).
                vector.wait_ge(tgt_sem, 16)
                vector.wait_ge(iota_sem, 1)
                for c in range(2, NCHUNK):
                    p0, p1 = PAIR_OFF[c], PAIR_OFF[c + 1]
                    np_ = p1 - p0
                    out_ap = bass.AP(
                        oh_t,
                        p0 * 2 * C,
                        [[NPAIR * 2 * C, P], [2 * C, np_], [C, 2], [1, C]],
                    )
                    iota_bc = bass.AP(
                        iota_t, 0, [[C, P], [0, np_], [0, 2], [1, C]]
                    )
                    tgt_bc = bass.AP(
                        tgt_t, 2 * p0, [[NB, P], [2, np_], [1, 2], [0, C]]
                    )
                    nc.vector.tensor_tensor(
                        out_ap,
                        iota_bc,
                        tgt_bc,
                        mybir.AluOpType.is_equal,
                    ).then_inc(oh_gen, 1)
                vector.wait_ge(pe_done, 1)
                # Two independent bank casts; each output DMA starts as soon
                # as its own cast completes (then_inc fires on completion).
                nc.vector.tensor_copy(out_t[:, 0:512], psum[:, 0:512]).then_inc(
                    vec_a, 1
                )
                nc.vector.tensor_copy(out_t[:, 512:1024], psum[:, 512:1024]).then_inc(
                    vec_b, 1
                )

    return nc


def _get_program() -> bass.Bass:
    global _PROGRAM
    if _PROGRAM is None:
        _PROGRAM = _build_program()
    return _PROGRAM


def _make_in_maps(modal1, modal2, targets):
    x1 = np.asarray(modal1, dtype=np.float32).astype(NPF8)
    x2 = np.asarray(modal2, dtype=np.float32).astype(NPF8)
    targets = np.asarray(targets)

    tgt_pb = np.ascontiguousarray(
        targets.reshape(NB, P).T.astype(np.float32)
    )  # [p, b] = targets[b*128+p]

    # [4096, D] -> [128, NB, D'] with [p, b] = sample b*128+p
    x1b = x1.reshape(NB, P, D).transpose(1, 0, 2)
    x2b = x2.reshape(NB, P, D).transpose(1, 0, 2)

    # head: one-hot for blocks 0..2*OH0_PAIRS-1 in [pr, i, c] layout
    oh0 = (
        tgt_pb[:, : 2 * OH0_PAIRS, None]
        == np.arange(C, dtype=np.float32)[None, None, :]
    ).astype(NPF8)

    in_maps = []
    for k in range(NCORES):
        sl = slice(k * DCHUNK, (k + 1) * DCHUNK)
        x = np.empty((P, XCOLS), dtype=NPF8)
        x[:, :HEADC] = oh0.reshape(P, HEADC)
        xb = x[:, HEADC:].reshape(P, NB, W)
        xb[:, :, :DCHUNK] = x1b[:, :, sl]
        xb[:, :, DCHUNK:] = x2b[:, :, sl]
        in_maps.append({"tgt": tgt_pb, "x": x})
    return in_maps


def _host_expected_sums(in_map):
    """Numpy model of the device output for one core (debug aid)."""
    tgt = in_map["tgt"].T.reshape(-1).astype(np.int64)  # sample order
    x = in_map["x"][:, HEADC:].astype(np.float32).reshape(P, NB, W)
    flat = x.transpose(1, 0, 2).reshape(N, W)
    out = np.zeros((C, 1024), np.float32)
    for b in range(NB):
        half = 0 if (b // 2) % 2 == 0 else 512
        blk = flat[b * P : (b + 1) * P]
        np.add.at(out[:, half : half + W], tgt[b * P : (b + 1) * P], blk)
    return out.astype(np.float16)


def _finish_on_host(sums_list, targets):
    """Form class Grams from the per-core segment sums and do the loss."""
    P1 = np.zeros((C, C), np.float64)
    P2 = np.zeros((C, C), np.float64)
    P3 = np.zeros((C, C), np.float64)
    for s in sums_list:
        s = np.asarray(s, np.float64)
        R = s[:, 0:256] + s[:, 512:768]      # [class, d-chunk]
        T = s[:, 256:512] + s[:, 768:1024]
        P1 += R @ R.T
        P2 += T @ T.T
        P3 += R @ T.T

    n = np.bincount(targets, minlength=C).astype(np.float64)
    u = 1.0 / np.maximum(n, 1.0)

    S_CC = P1 + P2 + P3 + P3.T  # (R+T)(R+T)^T
    uu = np.outer(u, u)
    A1 = 0.5 * uu * (P1 + P3)    # meanR . ctr
    A2 = 0.5 * uu * (P2 + P3.T)  # meanT . ctr
    nR = u * u * np.diag(P1)
    nT = u * u * np.diag(P2)
    nCtr = 0.25 * u * u * np.diag(S_CC)

    Wgt = np.outer(n, n)
    eye = np.eye(C)
    total = 0.0
    for A, nrm in ((A1, nR), (A2, nT)):
        sq = np.maximum(nrm[:, None] + nCtr[None, :] - 2.0 * A, 1e-12)
        d = np.sqrt(sq)
        dd = np.sqrt(d + 1e-10)
        term = eye * sq + (1.0 - eye) * np.maximum(MARGIN - dd, 0.0) ** 2
        total += (Wgt * term).sum() / (float(N) * float(N))
    return np.asarray(total, dtype=np.float32)


def kernel(modal1_inputs, modal2_inputs, targets):
    nc = _get_program()
    in_maps = _make_in_maps(modal1_inputs, modal2_inputs, targets)
    res = run_bass_kernel_spmd(nc, in_maps, list(range(NCORES)))
    sums_list = [res.results[k]["sums"] for k in range(NCORES)]
    return _finish_on_host(sums_list, np.asarray(targets))
